# revision 1
# baseline (speedup 1.0000x reference)
"""Causal multi-head attention (B=4, T=2048, D=1024, H=16) on 8 trn2 cores.

Sharding: core c -> (batch b = c//2, head-group g = c%2) -> 8 heads/core.
Per-core Bass kernel computes QKV projections, causal flash attention in
transposed-score orientation (s^T = K @ Q^T, softmax denominator via an
appended ones-column in V), and the head-sliced output projection partial.
Host sums the two head-group partials per batch (row-parallel proj).
"""

import numpy as np
import ml_dtypes

import concourse.bass as bass  # noqa: F401  (bass types via bacc)
import concourse.bacc as bacc
import concourse.mybir as mybir
import concourse.tile as tile
from concourse.bass_utils import run_bass_kernel_spmd

B, T, D = 4, 2048, 1024
H, DH = 16, 64
N_CORES = 8
HPC = 8      # heads per core
PAIRS = HPC // 2
BF = mybir.dt.bfloat16
F32 = mybir.dt.float32
BF_NP = ml_dtypes.bfloat16

TQ = 512     # q block (free dim)
TK = 128     # k block (partition dim)
NQG = T // TQ
NKC = T // TK


def build_nc():
    nc = bacc.Bacc(
        "TRN2",
        target_bir_lowering=False,
        debug=False,
        enable_asserts=True,
        num_devices=N_CORES,
    )
    xT = nc.dram_tensor("xT", [D, T], BF, kind="ExternalInput")
    wq = nc.dram_tensor("wq", [D, 512], BF, kind="ExternalInput")
    wk = nc.dram_tensor("wk", [D, 512], BF, kind="ExternalInput")
    wv = nc.dram_tensor("wv", [D, 512], BF, kind="ExternalInput")
    wp = nc.dram_tensor("wp", [512, D], BF, kind="ExternalInput")
    y = nc.dram_tensor("y", [T, D], F32, kind="ExternalOutput")

    with tile.TileContext(nc) as tc:
        with (
            tc.tile_pool(name="pers", bufs=1) as pers,
            tc.tile_pool(name="work", bufs=1) as work,
            tc.tile_pool(name="ps", bufs=1, space="PSUM") as pp,
        ):
            # ---- persistent SBUF ----
            xT_sb = pers.tile([128, 8, T], BF, tag="xT", name="xT_sb")
            wq_sb = pers.tile([128, 8, 512], BF, tag="wq", name="wq_sb")
            wk_sb = pers.tile([128, 8, 512], BF, tag="wk", name="wk_sb")
            wv_sb = pers.tile([128, 8, 512], BF, tag="wv", name="wv_sb")
            wp_sb = pers.tile([128, 4, D], BF, tag="wp", name="wp_sb")
            # V in token-major layout with a ones column per head: [tok, head, 65]
            vext = pers.tile([128, NKC, HPC, 65], BF, tag="vext", name="vext")
            # normalized attention outputs, d-major: [pair-chan, pair, tok]
            outT = pers.tile([128, PAIRS, T], BF, tag="outT", name="outT")
            # causal mask variants for diagonal blocks: keep q >= k + j*128
            mask_sb = pers.tile([128, 128], BF, tag="mask", name="mask_sb")

            # ---- loads ----
            for dc in range(8):
                nc.sync.dma_start(xT_sb[:, dc, :], xT[dc * 128:(dc + 1) * 128, :])
                nc.sync.dma_start(wq_sb[:, dc, :], wq[dc * 128:(dc + 1) * 128, :])
                nc.sync.dma_start(wk_sb[:, dc, :], wk[dc * 128:(dc + 1) * 128, :])
                nc.sync.dma_start(wv_sb[:, dc, :], wv[dc * 128:(dc + 1) * 128, :])
            for cc in range(4):
                nc.sync.dma_start(wp_sb[:, cc, :], wp[cc * 128:(cc + 1) * 128, :])
            nc.gpsimd.memset(vext[:, :, :, 64], 1.0)
            nc.gpsimd.memset(mask_sb[:, :], 1.0)
            nc.gpsimd.affine_select(
                mask_sb[:, :],
                mask_sb[:, :],
                pattern=[[1, 128]],
                compare_op=mybir.AluOpType.is_ge,
                fill=0.0,
                base=0,
                channel_multiplier=-1,
            )

            # ---- phase 1: V = x @ wv  (token-major, all heads at once) ----
            for tk in range(NKC):
                ps_v = pp.tile([128, 512], F32, tag="accQ", bufs=2, name="ps_v")
                for dc in range(8):
                    nc.tensor.matmul(
                        ps_v[:, :],
                        xT_sb[:, dc, tk * 128:(tk + 1) * 128],
                        wv_sb[:, dc, :],
                        start=(dc == 0),
                        stop=(dc == 7),
                    )
                nc.vector.tensor_copy(
                    vext[:, tk, :, 0:64],
                    ps_v.rearrange("p (h d) -> p h d", d=64),
                )

            # ---- phase 2: per head pair ----
            # QT/KT for q-group qg is produced just before the attention that
            # first needs it; the normalize chain of pair p is emitted inside
            # pair p+1's attention so its DVE/GPSIMD burst never blocks the
            # PE at a pair boundary.
            pending_norm = [None]

            def emit_norm():
                if pending_norm[0] is None:
                    return
                hp_, den_, outU_ = pending_norm[0]
                pending_norm[0] = None
                den_r = work.tile([128, 1024], F32, tag="denr", bufs=2,
                                  name="den_r")
                nc.vector.reciprocal(den_r[:, :], den_[:, :])
                for qg_ in range(NQG):
                    for h_ in (0, 1):
                        # partition_broadcast only reads base partition 0 on
                        # HW: stage the reciprocal row through partition 0
                        rc = work.tile([1, 512], F32, tag="rc", bufs=3,
                                       name="rc")
                        nc.vector.tensor_copy(
                            rc[0:1, :],
                            den_r[32 * qg_:32 * qg_ + 1,
                                  h_ * 512:(h_ + 1) * 512],
                        )
                        bc = work.tile([64, 512], F32, tag="bc", bufs=3,
                                       name="bc")
                        nc.gpsimd.partition_broadcast(bc[0:64, :], rc[0:1, :])
                        nc.vector.tensor_mul(
                            outT[h_ * 64:(h_ + 1) * 64, hp_,
                                 qg_ * TQ:(qg_ + 1) * TQ],
                            outU_[(qg_, h_)][0:64, :],
                            bc[0:64, :],
                        )

            for hp in range(PAIRS):
                qt = work.tile([128, T], BF, tag="qt", bufs=2, name="qt")
                kt = work.tile([128, T], BF, tag="kt", bufs=2, name="kt")
                den = work.tile([128, 1024], F32, tag="den", bufs=2, name="den")
                nc.gpsimd.memset(den[:, :], 1.0)
                outU = {}
                for qg in range(NQG):
                    # Q^T / K^T for this q-group, d-major
                    # (rows = pair channels: head0 0-63, head1 64-127)
                    ps_q = pp.tile([128, 512], F32, tag="accQ", bufs=2, name="ps_q")
                    ps_k = pp.tile([128, 512], F32, tag="accQ", bufs=2, name="ps_k")
                    for dc in range(8):
                        nc.tensor.matmul(
                            ps_q[:, :],
                            wq_sb[:, dc, hp * 128:(hp + 1) * 128],
                            xT_sb[:, dc, qg * TQ:(qg + 1) * TQ],
                            start=(dc == 0),
                            stop=(dc == 7),
                        )
                    for dc in range(8):
                        nc.tensor.matmul(
                            ps_k[:, :],
                            wk_sb[:, dc, hp * 128:(hp + 1) * 128],
                            xT_sb[:, dc, qg * TQ:(qg + 1) * TQ],
                            start=(dc == 0),
                            stop=(dc == 7),
                        )
                    nc.vector.tensor_copy(qt[:, qg * TQ:(qg + 1) * TQ], ps_q[:, :])
                    nc.vector.tensor_copy(kt[:, qg * TQ:(qg + 1) * TQ], ps_k[:, :])

                    # attention over k chunks 0..(qg+1)*4, software-pipelined:
                    # QK of chunk kc+1 is emitted before AV of chunk kc
                    psO0 = pp.tile([65, 512], F32, tag="accO", bufs=2, name="psO0")
                    psO1 = pp.tile([65, 512], F32, tag="accO", bufs=2, name="psO1")
                    kmax = (qg + 1) * (TQ // TK)
                    noff = qg * (TQ // TK)

                    def qk(kc):
                        # scores^T chunk for both heads: [k 128, q 512] x2
                        # on diagonal blocks only columns q >= j*128 are live
                        off = max(0, kc - noff) * TK
                        ps_s = pp.tile([128, 1024], F32, tag="sc", bufs=2, name="ps_s")
                        for h in (0, 1):
                            nc.tensor.matmul(
                                ps_s[:, h * 512 + off:(h + 1) * 512],
                                kt[h * 64:(h + 1) * 64, kc * TK:(kc + 1) * TK],
                                qt[h * 64:(h + 1) * 64, qg * TQ + off:(qg + 1) * TQ],
                                start=True, stop=True,
                            )
                        return ps_s

                    def softmax_av(kc, ps_s):
                        off = max(0, kc - noff) * TK
                        j = kc - noff
                        ex = work.tile([128, 1024], BF, tag="ex", bufs=6, name="ex")
                        for h, psO in ((0, psO0), (1, psO1)):
                            sl = slice(h * 512 + off, (h + 1) * 512)
                            nc.scalar.activation(
                                ex[:, sl], ps_s[:, sl],
                                mybir.ActivationFunctionType.Exp,
                            )
                            if j >= 0:
                                # causal mask on the diagonal 128x128 sub-block
                                msl = slice(h * 512 + off, h * 512 + off + TK)
                                nc.vector.tensor_mul(
                                    ex[:, msl], ex[:, msl], mask_sb[:, :]
                                )
                            nc.tensor.matmul(
                                psO[:, off:512],
                                vext[:, kc, hp * 2 + h, :],
                                ex[:, sl],
                                start=(kc == 0),
                                stop=(kc == kmax - 1),
                                skip_group_check=True,
                            )

                    prev = qk(0)
                    for kc in range(kmax):
                        nxt = qk(kc + 1) if kc + 1 < kmax else None
                        softmax_av(kc, prev)
                        prev = nxt

                    # evict unnormalized AV + denominator row to SBUF,
                    # freeing PSUM; stash denom rows for the batched recip
                    for h, psO in ((0, psO0), (1, psO1)):
                        oU = work.tile([65, 512], F32, tag="outU", bufs=16,
                                       name="oU")
                        nc.vector.tensor_copy(oU[:, :], psO[:, :])
                        nc.vector.tensor_copy(
                            den[32 * qg:32 * qg + 1, h * 512:(h + 1) * 512],
                            psO[64:65, :],
                        )
                        outU[(qg, h)] = oU

                    if qg == 0:
                        # previous pair's normalize lands here, overlapped
                        # with this pair's remaining attention
                        emit_norm()

                pending_norm[0] = (hp, den, outU)

            emit_norm()

            # ---- phase 3: y_partial = outT.T @ wp ----
            for tk in range(NKC):
                for nb in range(2):
                    ps_y = pp.tile([128, 512], F32, tag="accQ", bufs=2, name="ps_y")
                    for cc in range(4):
                        nc.tensor.matmul(
                            ps_y[:, :],
                            outT[:, cc, tk * 128:(tk + 1) * 128],
                            wp_sb[:, cc, nb * 512:(nb + 1) * 512],
                            start=(cc == 0),
                            stop=(cc == 3),
                        )
                    y_ev = work.tile([128, 512], F32, tag="yev", bufs=3, name="y_ev")
                    nc.scalar.copy(y_ev[:, :], ps_y[:, :])
                    nc.sync.dma_start(
                        y[tk * 128:(tk + 1) * 128, nb * 512:(nb + 1) * 512],
                        y_ev[:, :],
                    )

    nc.compile()
    return nc


_NC_CACHE = None


def _get_nc():
    global _NC_CACHE
    if _NC_CACHE is None:
        _NC_CACHE = build_nc()
    return _NC_CACHE


def make_in_maps(x, w_qkv, w_proj):
    """Host-side sharding: core c -> (batch c//2, head-group c%2)."""
    scale = np.float32(1.0 / np.sqrt(DH))
    in_maps = []
    for c in range(N_CORES):
        b, g = divmod(c, 2)
        sl = slice(g * 512, (g + 1) * 512)
        xT = np.ascontiguousarray(x[b].T).astype(BF_NP)
        wq = (w_qkv[:, 0 * D:1 * D][:, sl] * scale).astype(BF_NP)
        wk = w_qkv[:, 1 * D:2 * D][:, sl].astype(BF_NP)
        wv = w_qkv[:, 2 * D:3 * D][:, sl].astype(BF_NP)
        wp = np.ascontiguousarray(w_proj[sl, :]).astype(BF_NP)
        in_maps.append({"xT": xT, "wq": wq, "wk": wk, "wv": wv, "wp": wp})
    return in_maps


def kernel(x, w_qkv, w_proj, _trace=False, _tmpdir=None):
    x = np.asarray(x, dtype=np.float32)
    w_qkv = np.asarray(w_qkv, dtype=np.float32)
    w_proj = np.asarray(w_proj, dtype=np.float32)
    nc = _get_nc()
    in_maps = make_in_maps(x, w_qkv, w_proj)
    res = run_bass_kernel_spmd(
        nc, in_maps, core_ids=list(range(N_CORES)), trace=_trace, tmpdir=_tmpdir
    )
    out = np.empty((B, T, D), dtype=np.float32)
    for b in range(B):
        out[b] = res.results[2 * b]["y"] + res.results[2 * b + 1]["y"]
    if _trace:
        kernel._last_results = res
    return out



# revision 20
# speedup vs baseline: 1.0590x; 1.0590x over previous
"""Causal multi-head attention (B=4, T=2048, D=1024, H=16) on 8 trn2 cores.

Sharding: core c -> (batch b = c//2, head-group g = c%2) -> 8 heads/core.
Per-core Bass kernel: QKV projections, causal flash attention with
transposed scores (s^T = K @ Q^T) but q-major AV accumulation
(out[q, d] = ex^T V via ex-as-stationary matmuls, N=65 with an appended
ones column in V giving the softmax denominator per psum partition).
Normalization folds into PSUM eviction (reciprocal_approx_fast +
per-partition tensor_scalar_mul), then a PE transpose restores d-major
layout for the output projection. Host sums the two head-group partials
per batch (row-parallel proj).
"""

import numpy as np
import ml_dtypes

import concourse.bass as bass  # noqa: F401  (bass types via bacc)
import concourse.bacc as bacc
import concourse.mybir as mybir
import concourse.tile as tile
from concourse.bass_utils import run_bass_kernel_spmd

B, T, D = 4, 2048, 1024
H, DH = 16, 64
N_CORES = 8
HPC = 8      # heads per core
PAIRS = HPC // 2
BF = mybir.dt.bfloat16
F32 = mybir.dt.float32
BF_NP = ml_dtypes.bfloat16

TQ = 512     # q block (free dim)
TK = 128     # k block (partition dim)
NQG = T // TQ
NKC = T // TK
QB = TQ // TK   # 128-wide q sub-blocks per q group
EVICT_EARLY = True
EXACT_RECIP = False
DEBUG_OUTT = False


def build_nc():
    nc = bacc.Bacc(
        "TRN2",
        target_bir_lowering=False,
        debug=False,
        enable_asserts=True,
        num_devices=N_CORES,
    )
    xT = nc.dram_tensor("xT", [D, T], BF, kind="ExternalInput")
    wq = nc.dram_tensor("wq", [D, 512], BF, kind="ExternalInput")
    wk = nc.dram_tensor("wk", [D, 512], BF, kind="ExternalInput")
    wv = nc.dram_tensor("wv", [D, 512], BF, kind="ExternalInput")
    wp = nc.dram_tensor("wp", [512, D], BF, kind="ExternalInput")
    ident = nc.dram_tensor("ident", [128, 128], F32, kind="ExternalInput")
    y = nc.dram_tensor("y", [T, D], BF, kind="ExternalOutput")
    if DEBUG_OUTT:
        outT_dbg = nc.dram_tensor("outT_dbg", [128, PAIRS, T], BF,
                                  kind="ExternalOutput")
        o_dbg = nc.dram_tensor("o_dbg", [128, PAIRS, NQG, 2, QB, 64], F32,
                               kind="ExternalOutput")

    with tile.TileContext(nc) as tc:
        with (
            tc.tile_pool(name="pers", bufs=1) as pers,
            tc.tile_pool(name="work", bufs=1) as work,
            tc.tile_pool(name="ps", bufs=1, space="PSUM") as pp,
        ):
            # ---- persistent SBUF ----
            xT_sb = pers.tile([128, 8, T], BF, tag="xT", name="xT_sb")
            wq_sb = pers.tile([128, 8, 512], BF, tag="wq", name="wq_sb")
            wk_sb = pers.tile([128, 8, 512], BF, tag="wk", name="wk_sb")
            wv_sb = pers.tile([128, 8, 512], BF, tag="wv", name="wv_sb")
            wp_sb = pers.tile([128, 4, D], BF, tag="wp", name="wp_sb")
            id_f32 = pers.tile([128, 128], F32, tag="id", name="id_f32")
            # V in token-major layout with a ones column per head: [tok, head, 65]
            vext = pers.tile([128, NKC, HPC, 65], BF, tag="vext", name="vext")
            # normalized attention outputs, d-major: [pair-chan, pair, tok]
            outT = pers.tile([128, PAIRS, T], BF, tag="outT", name="outT")
            # causal mask for diagonal blocks: keep q >= k
            mask_sb = pers.tile([128, 128], BF, tag="mask", name="mask_sb")

            # ---- loads (phase-1 deps first) ----
            for dc in range(8):
                nc.sync.dma_start(xT_sb[:, dc, :], xT[dc * 128:(dc + 1) * 128, :])
                nc.sync.dma_start(wv_sb[:, dc, :], wv[dc * 128:(dc + 1) * 128, :])
            for dc in range(8):
                nc.sync.dma_start(wq_sb[:, dc, :], wq[dc * 128:(dc + 1) * 128, :])
                nc.sync.dma_start(wk_sb[:, dc, :], wk[dc * 128:(dc + 1) * 128, :])
            for cc in range(4):
                nc.sync.dma_start(wp_sb[:, cc, :], wp[cc * 128:(cc + 1) * 128, :])
            nc.sync.dma_start(id_f32[:, :], ident[:, :])
            nc.gpsimd.memset(vext[:, :, :, 64], 1.0)
            nc.gpsimd.memset(mask_sb[:, :], 1.0)
            nc.gpsimd.affine_select(
                mask_sb[:, :],
                mask_sb[:, :],
                pattern=[[1, 128]],
                compare_op=mybir.AluOpType.is_ge,
                fill=0.0,
                base=0,
                channel_multiplier=-1,
            )

            # ---- phase 1: V = x @ wv  (token-major, all heads at once) ----
            for tkk in range(NKC // 2):
                ps_v = pp.tile([128, 1024], F32, tag="sc", bufs=2, name="ps_v")
                for half in range(2):
                    for dc in range(8):
                        tk = tkk * 2 + half
                        nc.tensor.matmul(
                            ps_v[:, half * 512:(half + 1) * 512],
                            xT_sb[:, dc, tk * 128:(tk + 1) * 128],
                            wv_sb[:, dc, :],
                            start=(dc == 0),
                            stop=(dc == 7),
                            skip_group_check=True,
                        )
                for half in range(2):
                    tk = tkk * 2 + half
                    nc.vector.tensor_copy(
                        vext[:, tk, :, 0:64],
                        ps_v[:, half * 512:(half + 1) * 512].rearrange(
                            "p (h d) -> p h d", d=64),
                    )

            # ---- phase 2 ----
            pair_qt = {}
            pair_kt = {}

            def emit_proj(hp, qg):
                """Q^T / K^T for (pair hp, q-group qg), d-major.

                Rows = pair channels: head0 d 0-63 on partitions 0-63,
                head1 d 0-63 on partitions 64-127.
                """
                if qg == 0:
                    pair_qt[hp] = work.tile([128, T], BF, tag="qt", bufs=2,
                                            name="qt")
                    pair_kt[hp] = work.tile([128, T], BF, tag="kt", bufs=2,
                                            name="kt")
                qt, kt = pair_qt[hp], pair_kt[hp]
                ps_qk = pp.tile([128, 1024], F32, tag="sc", bufs=2,
                                name="ps_qk")
                for dc in range(8):
                    nc.tensor.matmul(
                        ps_qk[:, 0:512],
                        wq_sb[:, dc, hp * 128:(hp + 1) * 128],
                        xT_sb[:, dc, qg * TQ:(qg + 1) * TQ],
                        start=(dc == 0),
                        stop=(dc == 7),
                        skip_group_check=True,
                    )
                for dc in range(8):
                    nc.tensor.matmul(
                        ps_qk[:, 512:1024],
                        wk_sb[:, dc, hp * 128:(hp + 1) * 128],
                        xT_sb[:, dc, qg * TQ:(qg + 1) * TQ],
                        start=(dc == 0),
                        stop=(dc == 7),
                        skip_group_check=True,
                    )
                nc.vector.tensor_copy(qt[:, qg * TQ:(qg + 1) * TQ],
                                      ps_qk[:, 0:512])
                nc.vector.tensor_copy(kt[:, qg * TQ:(qg + 1) * TQ],
                                      ps_qk[:, 512:1024])

            emit_proj(0, 0)

            for hp in range(PAIRS):
                qt, kt = pair_qt[hp], pair_kt[hp]
                for qg in range(NQG):
                    kmax = (qg + 1) * QB
                    noff = qg * QB
                    # unnormalized AV accumulators, q-major:
                    # region (h, qb) = psO[h][:, qb, 0:64] + den col 64
                    psO = [
                        pp.tile([128, QB, 65], F32, tag=f"av{h}", bufs=1,
                                name=f"psO{h}")
                        for h in range(2)
                    ]
                    # transposed normalized outputs [d, qb, q] per head
                    psT = [
                        pp.tile([64, QB, 128], F32, tag=f"tp{h}", bufs=1,
                                name=f"psT{h}")
                        for h in range(2)
                    ]

                    def qk(kc):
                        # scores^T chunk for both heads: [k 128, q 512] x2
                        # on diagonal blocks only columns q >= j*128 live
                        off = max(0, kc - noff) * TK
                        ps_s = pp.tile([128, 1024], F32, tag="sc", bufs=2,
                                       name="ps_s")
                        for h in (0, 1):
                            nc.tensor.matmul(
                                ps_s[:, h * 512 + off:(h + 1) * 512],
                                kt[h * 64:(h + 1) * 64, kc * TK:(kc + 1) * TK],
                                qt[h * 64:(h + 1) * 64,
                                   qg * TQ + off:(qg + 1) * TQ],
                                start=True, stop=True,
                                skip_group_check=True,
                            )
                        return ps_s

                    def evict(h, qb):
                        # normalize region (h, qb) and transpose to psT
                        den_r = work.tile([128, 1], F32, tag="denr", bufs=4,
                                          name="den_r")
                        if EXACT_RECIP:
                            nc.vector.reciprocal(
                                den_r[:, :], psO[h][:, qb, 64:65])
                        else:
                            nc.vector.reciprocal_approx_fast(
                                den_r[:, :], psO[h][:, qb, 64:65])
                        o_sb = work.tile([128, 64], F32, tag="osb", bufs=4,
                                         name="o_sb")
                        nc.vector.tensor_scalar_mul(
                            o_sb[:, :], psO[h][:, qb, 0:64], den_r[:, :])
                        nc.tensor.transpose(
                            psT[h][0:64, qb, :], o_sb[:, :], id_f32[:, :])
                        if DEBUG_OUTT:
                            nc.sync.dma_start(
                                o_dbg[:, hp, qg, h, qb, :], o_sb[:, :])

                    def softmax_av(kc, ps_s):
                        off = max(0, kc - noff) * TK
                        j = kc - noff
                        ex = work.tile([128, 2, 512], BF, tag="ex", bufs=4,
                                       name="ex")
                        ps3 = ps_s.rearrange("p (h q) -> p h q", h=2)
                        nc.scalar.activation(
                            ex[:, :, off:], ps3[:, :, off:],
                            mybir.ActivationFunctionType.Exp,
                        )
                        if j >= 0:
                            # causal mask on the diagonal 128x128 sub-block
                            for h in (0, 1):
                                nc.vector.tensor_mul(
                                    ex[:, h, off:off + TK],
                                    ex[:, h, off:off + TK],
                                    mask_sb[:, :],
                                )
                        for h in (0, 1):
                            for qb in range(QB):
                                if j > qb:
                                    continue
                                # start=True clears has_written for the WHOLE
                                # psum bank, so only the first matmul per bank
                                # may set it; later first-writes of other
                                # regions overwrite via has_written=0.
                                nc.tensor.matmul(
                                    psO[h][:, qb, :],
                                    ex[:, h, qb * TK:(qb + 1) * TK],
                                    vext[:, kc, hp * 2 + h, :],
                                    start=(kc == 0 and qb == 0),
                                    stop=(kc == noff + qb),
                                    skip_group_check=True,
                                )
                        if j >= 0 and EVICT_EARLY:
                            for h in (0, 1):
                                evict(h, j)

                    prev = qk(0)
                    for kc in range(kmax):
                        if kc + 1 < kmax:
                            nxt = qk(kc + 1)
                        else:
                            nxt = None
                            # keep the PE fed during the AV tail
                            if qg + 1 < NQG:
                                emit_proj(hp, qg + 1)
                            elif hp + 1 < PAIRS:
                                emit_proj(hp + 1, 0)
                        softmax_av(kc, prev)
                        prev = nxt

                    if not EVICT_EARLY:
                        for j in range(QB):
                            for h in (0, 1):
                                evict(h, j)
                    for h in (0, 1):
                        nc.vector.tensor_copy(
                            outT[h * 64:(h + 1) * 64, hp,
                                 qg * TQ:(qg + 1) * TQ],
                            psT[h][0:64, :, :],
                        )

            if DEBUG_OUTT:
                for cc in range(4):
                    nc.sync.dma_start(outT_dbg[:, cc, :], outT[:, cc, :])

            # ---- phase 3: y_partial = outT.T @ wp ----
            for tk in range(NKC):
                ps_y = pp.tile([128, 1024], F32, tag="sc", bufs=2, name="ps_y")
                for nb in range(2):
                    for cc in range(4):
                        nc.tensor.matmul(
                            ps_y[:, nb * 512:(nb + 1) * 512],
                            outT[:, cc, tk * 128:(tk + 1) * 128],
                            wp_sb[:, cc, nb * 512:(nb + 1) * 512],
                            start=(cc == 0),
                            stop=(cc == 3),
                            skip_group_check=True,
                        )
                y_ev = work.tile([128, 1024], BF, tag="yev", bufs=3,
                                 name="y_ev")
                nc.scalar.copy(y_ev[:, :], ps_y[:, :])
                nc.sync.dma_start(
                    y[tk * 128:(tk + 1) * 128, :],
                    y_ev[:, :],
                )

    nc.compile()
    return nc


_NC_CACHE = None


def _get_nc():
    global _NC_CACHE
    if _NC_CACHE is None:
        _NC_CACHE = build_nc()
    return _NC_CACHE


def make_in_maps(x, w_qkv, w_proj):
    """Host-side sharding: core c -> (batch c//2, head-group c%2)."""
    scale = np.float32(1.0 / np.sqrt(DH))
    ident = np.eye(128, dtype=np.float32)
    in_maps = []
    for c in range(N_CORES):
        b, g = divmod(c, 2)
        sl = slice(g * 512, (g + 1) * 512)
        xT = np.ascontiguousarray(x[b].T).astype(BF_NP)
        wq = (w_qkv[:, 0 * D:1 * D][:, sl] * scale).astype(BF_NP)
        wk = w_qkv[:, 1 * D:2 * D][:, sl].astype(BF_NP)
        wv = w_qkv[:, 2 * D:3 * D][:, sl].astype(BF_NP)
        wp = np.ascontiguousarray(w_proj[sl, :]).astype(BF_NP)
        in_maps.append({"xT": xT, "wq": wq, "wk": wk, "wv": wv, "wp": wp,
                        "ident": ident})
    return in_maps


def kernel(x, w_qkv, w_proj, _trace=False, _tmpdir=None):
    x = np.asarray(x, dtype=np.float32)
    w_qkv = np.asarray(w_qkv, dtype=np.float32)
    w_proj = np.asarray(w_proj, dtype=np.float32)
    nc = _get_nc()
    in_maps = make_in_maps(x, w_qkv, w_proj)
    res = run_bass_kernel_spmd(
        nc, in_maps, core_ids=list(range(N_CORES)), trace=_trace, tmpdir=_tmpdir
    )
    out = np.empty((B, T, D), dtype=np.float32)
    for b in range(B):
        out[b] = (res.results[2 * b]["y"].astype(np.float32)
                  + res.results[2 * b + 1]["y"].astype(np.float32))
    if _trace:
        kernel._last_results = res
    return out


# revision 21
# speedup vs baseline: 1.0899x; 1.0292x over previous
"""Causal multi-head attention (B=4, T=2048, D=1024, H=16) on 8 trn2 cores.

Sharding: core c -> (batch b = c//2, head-group g = c%2) -> 8 heads/core.
Per-core Bass kernel: QKV projections, causal flash attention with
transposed scores (s^T = K @ Q^T) but q-major AV accumulation
(out[q, d] = ex^T V via ex-as-stationary matmuls, N=65 with an appended
ones column in V giving the softmax denominator per psum partition).
Normalization folds into PSUM eviction (reciprocal_approx_fast +
per-partition tensor_scalar_mul), then a PE transpose restores d-major
layout for the output projection. The attention inner loop is ACT
(exp) throughput bound, so V projection and Q/K projections are diced
into small matmul "pieces" pumped into the PE's slack between chunks.
Host sums the two head-group partials per batch (row-parallel proj).
"""

import numpy as np
import ml_dtypes

import concourse.bass as bass  # noqa: F401  (bass types via bacc)
import concourse.bacc as bacc
import concourse.mybir as mybir
import concourse.tile as tile
from concourse.bass_utils import run_bass_kernel_spmd

B, T, D = 4, 2048, 1024
H, DH = 16, 64
N_CORES = 8
HPC = 8      # heads per core
PAIRS = HPC // 2
BF = mybir.dt.bfloat16
F32 = mybir.dt.float32
BF_NP = ml_dtypes.bfloat16

TQ = 512     # q block (free dim)
TK = 128     # k block (partition dim)
NQG = T // TQ
NKC = T // TK
QB = TQ // TK   # 128-wide q sub-blocks per q group


def build_nc():
    nc = bacc.Bacc(
        "TRN2",
        target_bir_lowering=False,
        debug=False,
        enable_asserts=True,
        num_devices=N_CORES,
    )
    xT = nc.dram_tensor("xT", [D, T], BF, kind="ExternalInput")
    wq = nc.dram_tensor("wq", [D, 512], BF, kind="ExternalInput")
    wk = nc.dram_tensor("wk", [D, 512], BF, kind="ExternalInput")
    wv = nc.dram_tensor("wv", [D, 512], BF, kind="ExternalInput")
    wp = nc.dram_tensor("wp", [512, D], BF, kind="ExternalInput")
    ident = nc.dram_tensor("ident", [128, 128], BF, kind="ExternalInput")
    y = nc.dram_tensor("y", [T, D], BF, kind="ExternalOutput")

    with tile.TileContext(nc) as tc:
        with (
            tc.tile_pool(name="pers", bufs=1) as pers,
            tc.tile_pool(name="work", bufs=1) as work,
            tc.tile_pool(name="ps", bufs=1, space="PSUM") as pp,
        ):
            # ---- persistent SBUF (per-dc tiles => DMA-granular deps) ----
            xT_t = [pers.tile([128, T], BF, tag=f"xT{dc}", name=f"xT{dc}")
                    for dc in range(8)]
            wq_t = [pers.tile([128, 512], BF, tag=f"wq{dc}", name=f"wq{dc}")
                    for dc in range(8)]
            wk_t = [pers.tile([128, 512], BF, tag=f"wk{dc}", name=f"wk{dc}")
                    for dc in range(8)]
            wv_t = [pers.tile([128, 512], BF, tag=f"wv{dc}", name=f"wv{dc}")
                    for dc in range(8)]
            wp_sb = pers.tile([128, 4, D], BF, tag="wp", name="wp_sb")
            id_sb = pers.tile([128, 128], BF, tag="id", name="id_sb")
            # V in token-major layout with a ones column per head: [tok, head, 65]
            vext = pers.tile([128, NKC, HPC, 65], BF, tag="vext", name="vext")
            # normalized attention outputs, d-major: [pair-chan, pair, tok]
            outT = pers.tile([128, PAIRS, T], BF, tag="outT", name="outT")
            # causal mask for diagonal blocks: keep q >= k
            mask_sb = pers.tile([128, 128], BF, tag="mask", name="mask_sb")

            # ---- loads, chunk-interleaved so compute starts early ----
            nc.sync.dma_start(id_sb[:, :], ident[:, :])
            for dc in range(8):
                nc.sync.dma_start(wq_t[dc][:, :], wq[dc * 128:(dc + 1) * 128, :])
                nc.sync.dma_start(wk_t[dc][:, :], wk[dc * 128:(dc + 1) * 128, :])
                nc.sync.dma_start(xT_t[dc][:, :], xT[dc * 128:(dc + 1) * 128, :])
                nc.sync.dma_start(wv_t[dc][:, :], wv[dc * 128:(dc + 1) * 128, :])
            for cc in range(4):
                nc.sync.dma_start(wp_sb[:, cc, :], wp[cc * 128:(cc + 1) * 128, :])
            nc.gpsimd.memset(vext[:, :, :, 64], 1.0)
            nc.gpsimd.memset(mask_sb[:, :], 1.0)
            nc.gpsimd.affine_select(
                mask_sb[:, :],
                mask_sb[:, :],
                pattern=[[1, 128]],
                compare_op=mybir.AluOpType.is_ge,
                fill=0.0,
                base=0,
                channel_multiplier=-1,
            )

            # ---- background work pieces (V proj, Q/K proj) ----
            pair_qt = {}
            pair_kt = {}
            v_done = set()
            proj_done = set()

            def emit_v(tk):
                """V chunk tk: vext[:, tk] = (x @ wv)[tk block], + ones col."""
                if tk in v_done:
                    return
                v_done.add(tk)
                ps_v = pp.tile([128, 512], F32, tag="pq", bufs=1, name="ps_v")
                for dc in range(8):
                    nc.tensor.matmul(
                        ps_v[:, :],
                        xT_t[dc][:, tk * 128:(tk + 1) * 128],
                        wv_t[dc][:, :],
                        start=(dc == 0),
                        stop=(dc == 7),
                        skip_group_check=True,
                    )
                nc.vector.tensor_copy(
                    vext[:, tk, :, 0:64],
                    ps_v.rearrange("p (h d) -> p h d", d=64),
                )

            def emit_proj(hp, qg, which):
                """Q^T or K^T for (pair hp, q-group qg), d-major.

                Rows = pair channels: head0 d 0-63 on partitions 0-63,
                head1 d 0-63 on partitions 64-127.
                """
                if (hp, qg, which) in proj_done:
                    return
                proj_done.add((hp, qg, which))
                if hp not in pair_qt:
                    pair_qt[hp] = work.tile([128, T], BF, tag="qt", bufs=2,
                                            name="qt")
                    pair_kt[hp] = work.tile([128, T], BF, tag="kt", bufs=2,
                                            name="kt")
                dst = pair_qt[hp] if which == "q" else pair_kt[hp]
                w_t = wq_t if which == "q" else wk_t
                ps_p = pp.tile([128, 512], F32, tag="pq", bufs=1, name="ps_p")
                for dc in range(8):
                    nc.tensor.matmul(
                        ps_p[:, :],
                        w_t[dc][:, hp * 128:(hp + 1) * 128],
                        xT_t[dc][:, qg * TQ:(qg + 1) * TQ],
                        start=(dc == 0),
                        stop=(dc == 7),
                        skip_group_check=True,
                    )
                nc.vector.tensor_copy(dst[:, qg * TQ:(qg + 1) * TQ], ps_p[:, :])

            queue = []

            def pump(n):
                for _ in range(min(n, len(queue))):
                    queue.pop(0)()

            # ---- phase 2 ----
            emit_proj(0, 0, "q")
            emit_proj(0, 0, "k")
            for tk in range(QB):
                emit_v(tk)

            for hp in range(PAIRS):
                for qg in range(NQG):
                    kmax = (qg + 1) * QB
                    noff = qg * QB
                    # overdue pieces first (deps of this q-group)
                    pump(len(queue))
                    emit_proj(hp, qg, "q")
                    emit_proj(hp, qg, "k")
                    if hp == 0:
                        for tk in range(kmax):
                            emit_v(tk)
                    # enqueue pieces for the next q-group
                    if qg + 1 < NQG:
                        nhp, nqg = hp, qg + 1
                    elif hp + 1 < PAIRS:
                        nhp, nqg = hp + 1, 0
                    else:
                        nhp = None
                    if nhp is not None:
                        queue.append(lambda a=nhp, b=nqg: emit_proj(a, b, "q"))
                        queue.append(lambda a=nhp, b=nqg: emit_proj(a, b, "k"))
                        if nhp == 0:
                            for tk in range(kmax, (nqg + 1) * QB):
                                queue.append(lambda t=tk: emit_v(t))

                    qt, kt = pair_qt[hp], pair_kt[hp]
                    # unnormalized AV accumulators, q-major:
                    # region (h, qb) = psO[h][:, qb, 0:64] + den col 64
                    psO = [
                        pp.tile([128, QB, 65], F32, tag=f"av{h}", bufs=1,
                                name=f"psO{h}")
                        for h in range(2)
                    ]
                    # transposed normalized outputs [d, (h qb), q]
                    psT = pp.tile([64, 2 * QB, 128], BF, tag="tp", bufs=1,
                                  name="psT")

                    def qk(kc):
                        # scores^T chunk for both heads: [k 128, q 512] x2
                        # on diagonal blocks only columns q >= j*128 live
                        off = max(0, kc - noff) * TK
                        ps_s = pp.tile([128, 1024], F32, tag="sc", bufs=2,
                                       name="ps_s")
                        for h in (0, 1):
                            nc.tensor.matmul(
                                ps_s[:, h * 512 + off:(h + 1) * 512],
                                kt[h * 64:(h + 1) * 64, kc * TK:(kc + 1) * TK],
                                qt[h * 64:(h + 1) * 64,
                                   qg * TQ + off:(qg + 1) * TQ],
                                start=True, stop=True,
                                skip_group_check=True,
                            )
                        return ps_s

                    def evict(h, qb):
                        # normalize region (h, qb) and transpose to psT
                        den_r = work.tile([128, 1], F32, tag="denr", bufs=4,
                                          name="den_r")
                        nc.vector.reciprocal_approx_fast(
                            den_r[:, :], psO[h][:, qb, 64:65])
                        o_sb = work.tile([128, 64], BF, tag="osb", bufs=4,
                                         name="o_sb")
                        nc.vector.tensor_scalar_mul(
                            o_sb[:, :], psO[h][:, qb, 0:64], den_r[:, :])
                        nc.tensor.transpose(
                            psT[0:64, h * QB + qb, :], o_sb[:, :], id_sb[:, :])

                    def softmax_av(kc, ps_s):
                        off = max(0, kc - noff) * TK
                        j = kc - noff
                        ex = work.tile([128, 2, 512], BF, tag="ex", bufs=4,
                                       name="ex")
                        ps3 = ps_s.rearrange("p (h q) -> p h q", h=2)
                        nc.scalar.activation(
                            ex[:, :, off:], ps3[:, :, off:],
                            mybir.ActivationFunctionType.Exp,
                        )
                        if j >= 0:
                            # causal mask on the diagonal 128x128 sub-block
                            for h in (0, 1):
                                nc.vector.tensor_mul(
                                    ex[:, h, off:off + TK],
                                    ex[:, h, off:off + TK],
                                    mask_sb[:, :],
                                )
                        for h in (0, 1):
                            for qb in range(QB):
                                if j > qb:
                                    continue
                                # start=True clears has_written for the WHOLE
                                # psum bank, so only the first matmul per bank
                                # may set it; later first-writes of other
                                # regions overwrite via has_written=0.
                                nc.tensor.matmul(
                                    psO[h][:, qb, :],
                                    ex[:, h, qb * TK:(qb + 1) * TK],
                                    vext[:, kc, hp * 2 + h, :],
                                    start=(kc == 0 and qb == 0),
                                    stop=(kc == noff + qb),
                                    skip_group_check=True,
                                )
                        if j >= 0:
                            for h in (0, 1):
                                evict(h, j)

                    prev = qk(0)
                    for kc in range(kmax):
                        nxt = qk(kc + 1) if kc + 1 < kmax else None
                        softmax_av(kc, prev)
                        pump(1)
                        prev = nxt

                    for h in (0, 1):
                        nc.vector.tensor_copy(
                            outT[h * 64:(h + 1) * 64, hp,
                                 qg * TQ:(qg + 1) * TQ],
                            psT[0:64, h * QB:(h + 1) * QB, :],
                        )

            # ---- phase 3: y_partial = outT.T @ wp ----
            for tk in range(NKC):
                ps_y = pp.tile([128, 1024], F32, tag="sc", bufs=2, name="ps_y")
                for nb in range(2):
                    for cc in range(4):
                        nc.tensor.matmul(
                            ps_y[:, nb * 512:(nb + 1) * 512],
                            outT[:, cc, tk * 128:(tk + 1) * 128],
                            wp_sb[:, cc, nb * 512:(nb + 1) * 512],
                            start=(cc == 0),
                            stop=(cc == 3),
                            skip_group_check=True,
                        )
                y_ev = work.tile([128, 1024], BF, tag="yev", bufs=3,
                                 name="y_ev")
                nc.scalar.copy(y_ev[:, :], ps_y[:, :])
                nc.sync.dma_start(
                    y[tk * 128:(tk + 1) * 128, :],
                    y_ev[:, :],
                )

    nc.compile()
    return nc


_NC_CACHE = None


def _get_nc():
    global _NC_CACHE
    if _NC_CACHE is None:
        _NC_CACHE = build_nc()
    return _NC_CACHE


def make_in_maps(x, w_qkv, w_proj):
    """Host-side sharding: core c -> (batch c//2, head-group c%2)."""
    scale = np.float32(1.0 / np.sqrt(DH))
    ident = np.eye(128, dtype=BF_NP)
    in_maps = []
    for c in range(N_CORES):
        b, g = divmod(c, 2)
        sl = slice(g * 512, (g + 1) * 512)
        xT = np.ascontiguousarray(x[b].T).astype(BF_NP)
        wq = (w_qkv[:, 0 * D:1 * D][:, sl] * scale).astype(BF_NP)
        wk = w_qkv[:, 1 * D:2 * D][:, sl].astype(BF_NP)
        wv = w_qkv[:, 2 * D:3 * D][:, sl].astype(BF_NP)
        wp = np.ascontiguousarray(w_proj[sl, :]).astype(BF_NP)
        in_maps.append({"xT": xT, "wq": wq, "wk": wk, "wv": wv, "wp": wp,
                        "ident": ident})
    return in_maps


def kernel(x, w_qkv, w_proj, _trace=False, _tmpdir=None):
    x = np.asarray(x, dtype=np.float32)
    w_qkv = np.asarray(w_qkv, dtype=np.float32)
    w_proj = np.asarray(w_proj, dtype=np.float32)
    nc = _get_nc()
    in_maps = make_in_maps(x, w_qkv, w_proj)
    res = run_bass_kernel_spmd(
        nc, in_maps, core_ids=list(range(N_CORES)), trace=_trace, tmpdir=_tmpdir
    )
    out = np.empty((B, T, D), dtype=np.float32)
    for b in range(B):
        out[b] = (res.results[2 * b]["y"].astype(np.float32)
                  + res.results[2 * b + 1]["y"].astype(np.float32))
    if _trace:
        kernel._last_results = res
    return out


# revision 36
# speedup vs baseline: 1.2844x; 1.1784x over previous
"""Causal multi-head attention (B=4, T=2048, D=1024, H=16) on 8 trn2 cores.

Sharding: core c -> (batch b = c//2, head-group g = c%2) -> 8 heads/core.
Per-core Bass kernel: QKV projections, causal flash attention with
transposed scores (s^T = K @ Q^T) but q-major AV accumulation
(out[q, d] = ex^T V via ex-as-stationary matmuls, N=65 with an appended
ones column in V giving the softmax denominator per psum partition).
Normalization folds into PSUM eviction (reciprocal_approx_fast +
per-partition tensor_scalar_mul), then a PE transpose restores d-major
layout for the output projection. The attention inner loop is ACT
(exp) throughput bound, so V projection and Q/K projections are diced
into small matmul "pieces" pumped into the PE's slack between chunks.
Host sums the two head-group partials per batch (row-parallel proj).
"""

import numpy as np
import ml_dtypes

import concourse.bass as bass  # noqa: F401  (bass types via bacc)
import concourse.bacc as bacc
import concourse.mybir as mybir
import concourse.tile as tile
from concourse.bass_utils import run_bass_kernel_spmd

B, T, D = 4, 2048, 1024
H, DH = 16, 64
N_CORES = 8
HPC = 8      # heads per core
PAIRS = HPC // 2
BF = mybir.dt.bfloat16
F32 = mybir.dt.float32
BF_NP = ml_dtypes.bfloat16

TQ = 512     # q block (free dim)
TK = 128     # k block (partition dim)
NQG = T // TQ
NKC = T // TK
QB = TQ // TK   # 128-wide q sub-blocks per q group


def build_nc():
    nc = bacc.Bacc(
        "TRN2",
        target_bir_lowering=False,
        debug=False,
        enable_asserts=True,
        num_devices=N_CORES,
    )
    xT = nc.dram_tensor("xT", [D, T], BF, kind="ExternalInput")
    wq = nc.dram_tensor("wq", [D, 512], BF, kind="ExternalInput")
    wk = nc.dram_tensor("wk", [D, 512], BF, kind="ExternalInput")
    wv = nc.dram_tensor("wv", [D, 512], BF, kind="ExternalInput")
    wp = nc.dram_tensor("wp", [512, D], BF, kind="ExternalInput")
    ident = nc.dram_tensor("ident", [128, 128], BF, kind="ExternalInput")
    y = nc.dram_tensor("y", [T, D], BF, kind="ExternalOutput")

    with tile.TileContext(nc) as tc:
        with (
            tc.tile_pool(name="pers", bufs=1) as pers,
            tc.tile_pool(name="work", bufs=1) as work,
            tc.tile_pool(name="ps", bufs=1, space="PSUM") as pp,
        ):
            # ---- persistent SBUF (per-dc tiles => DMA-granular deps) ----
            xT_t = [pers.tile([128, T], BF, tag=f"xT{dc}", name=f"xT{dc}")
                    for dc in range(8)]
            wq_t = [pers.tile([128, 512], BF, tag=f"wq{dc}", name=f"wq{dc}")
                    for dc in range(8)]
            wk_t = [pers.tile([128, 512], BF, tag=f"wk{dc}", name=f"wk{dc}")
                    for dc in range(8)]
            wv_t = [pers.tile([128, 512], BF, tag=f"wv{dc}", name=f"wv{dc}")
                    for dc in range(8)]
            wp_sb = pers.tile([128, 4, D], BF, tag="wp", name="wp_sb")
            id_sb = pers.tile([128, 128], BF, tag="id", name="id_sb")
            # V in token-major layout with a ones column per head: [tok, head, 65]
            vext = pers.tile([128, NKC, HPC, 65], BF, tag="vext", name="vext")
            # normalized attention outputs, d-major: [pair-chan, pair, tok]
            outT = pers.tile([128, PAIRS, T], BF, tag="outT", name="outT")
            # causal mask for diagonal blocks: keep q >= k
            mask_sb = pers.tile([128, 128], BF, tag="mask", name="mask_sb")

            # ---- loads, chunk-interleaved so compute starts early ----
            nc.sync.dma_start(id_sb[:, :], ident[:, :])
            for dc in range(8):
                nc.sync.dma_start(wq_t[dc][:, :], wq[dc * 128:(dc + 1) * 128, :])
                nc.sync.dma_start(wk_t[dc][:, :], wk[dc * 128:(dc + 1) * 128, :])
                nc.sync.dma_start(xT_t[dc][:, :], xT[dc * 128:(dc + 1) * 128, :])
                nc.sync.dma_start(wv_t[dc][:, :], wv[dc * 128:(dc + 1) * 128, :])
            for cc in range(4):
                nc.sync.dma_start(wp_sb[:, cc, :], wp[cc * 128:(cc + 1) * 128, :])
            nc.gpsimd.memset(vext[:, :, :, 64], 1.0)
            nc.gpsimd.memset(mask_sb[:, :], 1.0)
            nc.gpsimd.affine_select(
                mask_sb[:, :],
                mask_sb[:, :],
                pattern=[[1, 128]],
                compare_op=mybir.AluOpType.is_ge,
                fill=0.0,
                base=0,
                channel_multiplier=-1,
            )

            # ---- background work pieces (V proj, Q/K proj) ----
            pair_qt = {}
            pair_kt = {}
            v_done = set()
            proj_done = set()

            def emit_v(tk):
                """V chunk tk: vext[:, tk] = (x @ wv)[tk block], + ones col."""
                if tk in v_done:
                    return
                v_done.add(tk)
                ps_v = pp.tile([128, 512], F32, tag="pq", bufs=1, name="ps_v")
                for dc in range(8):
                    nc.tensor.matmul(
                        ps_v[:, :],
                        xT_t[dc][:, tk * 128:(tk + 1) * 128],
                        wv_t[dc][:, :],
                        start=(dc == 0),
                        stop=(dc == 7),
                        skip_group_check=True,
                    )
                nc.vector.tensor_copy(
                    vext[:, tk, :, 0:64],
                    ps_v.rearrange("p (h d) -> p h d", d=64),
                )

            def emit_proj(hp, qg, which):
                """Q^T or K^T for (pair hp, q-group qg), d-major.

                Rows = pair channels: head0 d 0-63 on partitions 0-63,
                head1 d 0-63 on partitions 64-127.
                """
                if (hp, qg, which) in proj_done:
                    return
                proj_done.add((hp, qg, which))
                if hp not in pair_qt:
                    pair_qt[hp] = work.tile([128, T], BF, tag="qt", bufs=2,
                                            name="qt")
                    pair_kt[hp] = work.tile([128, T], BF, tag="kt", bufs=2,
                                            name="kt")
                dst = pair_qt[hp] if which == "q" else pair_kt[hp]
                w_t = wq_t if which == "q" else wk_t
                ps_p = pp.tile([128, 512], F32, tag="pq", bufs=1, name="ps_p")
                for dc in range(8):
                    nc.tensor.matmul(
                        ps_p[:, :],
                        w_t[dc][:, hp * 128:(hp + 1) * 128],
                        xT_t[dc][:, qg * TQ:(qg + 1) * TQ],
                        start=(dc == 0),
                        stop=(dc == 7),
                        skip_group_check=True,
                    )
                nc.vector.tensor_copy(dst[:, qg * TQ:(qg + 1) * TQ], ps_p[:, :])

            queue = []

            def pump(n):
                for _ in range(min(n, len(queue))):
                    queue.pop(0)()

            # ---- phase 2 ----
            emit_proj(0, 0, "q")
            emit_proj(0, 0, "k")

            for hp in range(PAIRS):
                for qg in range(NQG):
                    kmax = (qg + 1) * QB
                    noff = qg * QB
                    # overdue pieces first (deps of this q-group)
                    pump(len(queue))
                    emit_proj(hp, qg, "q")
                    emit_proj(hp, qg, "k")
                    # enqueue pieces for the next q-group
                    if qg + 1 < NQG:
                        nhp, nqg = hp, qg + 1
                    elif hp + 1 < PAIRS:
                        nhp, nqg = hp + 1, 0
                    else:
                        nhp = None
                    if nhp is not None:
                        queue.append(lambda a=nhp, b=nqg: emit_proj(a, b, "q"))
                        queue.append(lambda a=nhp, b=nqg: emit_proj(a, b, "k"))
                        if nhp == 0:
                            for tk in range(kmax, (nqg + 1) * QB):
                                queue.append(lambda t=tk: emit_v(t))

                    qt, kt = pair_qt[hp], pair_kt[hp]
                    # unnormalized AV accumulators, q-major:
                    # region (h, qb) = psO[h][:, qb, 0:64] + den col 64
                    psO = [
                        pp.tile([128, QB, 65], F32, tag=f"av{h}", bufs=1,
                                name=f"psO{h}")
                        for h in range(2)
                    ]
                    # transposed normalized outputs [d, (h qb), q]
                    psT = pp.tile([64, 2 * QB, 128], BF, tag="tp", bufs=1,
                                  name="psT")

                    def qk(kc):
                        # scores^T chunk for both heads: [k 128, q 512] x2
                        # on diagonal blocks only columns q >= j*128 live
                        off = max(0, kc - noff) * TK
                        ps_s = pp.tile([128, 1024], F32, tag="sc", bufs=2,
                                       name="ps_s")
                        for h in (0, 1):
                            nc.tensor.matmul(
                                ps_s[:, h * 512 + off:(h + 1) * 512],
                                kt[h * 64:(h + 1) * 64, kc * TK:(kc + 1) * TK],
                                qt[h * 64:(h + 1) * 64,
                                   qg * TQ + off:(qg + 1) * TQ],
                                start=True, stop=True,
                                skip_group_check=True,
                            )
                        return ps_s

                    def evict(h, qb):
                        # normalize region (h, qb) and transpose to psT
                        den_r = work.tile([128, 1], F32, tag="denr", bufs=4,
                                          name="den_r")
                        nc.vector.reciprocal_approx_fast(
                            den_r[:, :], psO[h][:, qb, 64:65])
                        o_sb = work.tile([128, 64], BF, tag="osb", bufs=4,
                                         name="o_sb")
                        nc.vector.tensor_scalar_mul(
                            o_sb[:, :], psO[h][:, qb, 0:64], den_r[:, :])
                        nc.tensor.transpose(
                            psT[0:64, h * QB + qb, :], o_sb[:, :], id_sb[:, :])

                    def softmax_av(kc, ps_s):
                        off = max(0, kc - noff) * TK
                        j = kc - noff
                        ex = work.tile([128, 2, 512], BF, tag="ex", bufs=4,
                                       name="ex")
                        ps3 = ps_s.rearrange("p (h q) -> p h q", h=2)
                        nc.scalar.activation(
                            ex[:, :, off:], ps3[:, :, off:],
                            mybir.ActivationFunctionType.Exp,
                        )
                        if hp == 0:
                            # just-in-time V chunks, one chunk ahead
                            emit_v(kc)
                            if kc + 1 < kmax:
                                emit_v(kc + 1)
                        if j >= 0:
                            # causal mask on the diagonal 128x128 sub-block
                            for h in (0, 1):
                                nc.vector.tensor_mul(
                                    ex[:, h, off:off + TK],
                                    ex[:, h, off:off + TK],
                                    mask_sb[:, :],
                                )
                        for h in (0, 1):
                            for qb in range(QB):
                                if j > qb:
                                    continue
                                # start=True clears has_written for the WHOLE
                                # psum bank, so only the first matmul per bank
                                # may set it; later first-writes of other
                                # regions overwrite via has_written=0.
                                nc.tensor.matmul(
                                    psO[h][:, qb, :],
                                    ex[:, h, qb * TK:(qb + 1) * TK],
                                    vext[:, kc, hp * 2 + h, :],
                                    start=(kc == 0 and qb == 0),
                                    stop=(kc == noff + qb),
                                    skip_group_check=True,
                                )
                        if j >= 0:
                            for h in (0, 1):
                                evict(h, j)

                    prev = qk(0)
                    for kc in range(kmax):
                        nxt = qk(kc + 1) if kc + 1 < kmax else None
                        softmax_av(kc, prev)
                        pump(1)
                        prev = nxt

                    for h in (0, 1):
                        nc.vector.tensor_copy(
                            outT[h * 64:(h + 1) * 64, hp,
                                 qg * TQ:(qg + 1) * TQ],
                            psT[0:64, h * QB:(h + 1) * QB, :],
                        )

            # ---- phase 3: y_partial = outT.T @ wp ----
            for tk in range(NKC):
                ps_y = pp.tile([128, 1024], F32, tag="sc", bufs=2, name="ps_y")
                for nb in range(2):
                    for cc in range(4):
                        nc.tensor.matmul(
                            ps_y[:, nb * 512:(nb + 1) * 512],
                            outT[:, cc, tk * 128:(tk + 1) * 128],
                            wp_sb[:, cc, nb * 512:(nb + 1) * 512],
                            start=(cc == 0),
                            stop=(cc == 3),
                            skip_group_check=True,
                        )
                y_ev = work.tile([128, 1024], BF, tag="yev", bufs=3,
                                 name="y_ev")
                nc.vector.tensor_copy(y_ev[:, :], ps_y[:, :])
                nc.sync.dma_start(
                    y[tk * 128:(tk + 1) * 128, :],
                    y_ev[:, :],
                )

    nc.compile()
    return nc


_NC_CACHE = None


def _get_nc():
    global _NC_CACHE
    if _NC_CACHE is None:
        _NC_CACHE = build_nc()
    return _NC_CACHE


def make_in_maps(x, w_qkv, w_proj):
    """Host-side sharding: core c -> (batch c//2, head-group c%2)."""
    scale = np.float32(1.0 / np.sqrt(DH))
    ident = np.eye(128, dtype=BF_NP)
    in_maps = []
    for c in range(N_CORES):
        b, g = divmod(c, 2)
        sl = slice(g * 512, (g + 1) * 512)
        xT = np.ascontiguousarray(x[b].T).astype(BF_NP)
        wq = (w_qkv[:, 0 * D:1 * D][:, sl] * scale).astype(BF_NP)
        wk = w_qkv[:, 1 * D:2 * D][:, sl].astype(BF_NP)
        wv = w_qkv[:, 2 * D:3 * D][:, sl].astype(BF_NP)
        wp = np.ascontiguousarray(w_proj[sl, :]).astype(BF_NP)
        in_maps.append({"xT": xT, "wq": wq, "wk": wk, "wv": wv, "wp": wp,
                        "ident": ident})
    return in_maps


def kernel(x, w_qkv, w_proj, _trace=False, _tmpdir=None):
    x = np.asarray(x, dtype=np.float32)
    w_qkv = np.asarray(w_qkv, dtype=np.float32)
    w_proj = np.asarray(w_proj, dtype=np.float32)
    nc = _get_nc()
    in_maps = make_in_maps(x, w_qkv, w_proj)
    res = run_bass_kernel_spmd(
        nc, in_maps, core_ids=list(range(N_CORES)), trace=_trace, tmpdir=_tmpdir
    )
    out = np.empty((B, T, D), dtype=np.float32)
    for b in range(B):
        out[b] = (res.results[2 * b]["y"].astype(np.float32)
                  + res.results[2 * b + 1]["y"].astype(np.float32))
    if _trace:
        kernel._last_results = res
    return out


# revision 41
# speedup vs baseline: 1.3025x; 1.0141x over previous
"""Causal multi-head attention (B=4, T=2048, D=1024, H=16) on 8 trn2 cores.

Sharding: core c -> (batch b = c//2, head-group g = c%2) -> 8 heads/core.
Per-core Bass kernel: QKV projections, causal flash attention with
transposed scores (s^T = K @ Q^T) but q-major AV accumulation
(out[q, d] = ex^T V via ex-as-stationary matmuls, N=65 with an appended
ones column in V giving the softmax denominator per psum partition).
Normalization folds into PSUM eviction (reciprocal_approx_fast +
per-partition tensor_scalar_mul), then a PE transpose restores d-major
layout for the output projection. The attention inner loop is ACT
(exp) throughput bound, so V projection and Q/K projections are diced
into small matmul "pieces" pumped into the PE's slack between chunks.
Host sums the two head-group partials per batch (row-parallel proj).
"""

import numpy as np
import ml_dtypes

import concourse.bass as bass  # noqa: F401  (bass types via bacc)
import concourse.bacc as bacc
import concourse.mybir as mybir
import concourse.tile as tile
from concourse.bass_utils import run_bass_kernel_spmd

B, T, D = 4, 2048, 1024
H, DH = 16, 64
N_CORES = 8
HPC = 8      # heads per core
PAIRS = HPC // 2
BF = mybir.dt.bfloat16
F32 = mybir.dt.float32
BF_NP = ml_dtypes.bfloat16

TQ = 512     # q block (free dim)
TK = 128     # k block (partition dim)
NQG = T // TQ
NKC = T // TK
QB = TQ // TK   # 128-wide q sub-blocks per q group


def build_nc():
    nc = bacc.Bacc(
        "TRN2",
        target_bir_lowering=False,
        debug=False,
        enable_asserts=True,
        num_devices=N_CORES,
    )
    xT = nc.dram_tensor("xT", [D, T], BF, kind="ExternalInput")
    wq = nc.dram_tensor("wq", [D, 512], BF, kind="ExternalInput")
    wk = nc.dram_tensor("wk", [D, 512], BF, kind="ExternalInput")
    wv = nc.dram_tensor("wv", [D, 512], BF, kind="ExternalInput")
    wp = nc.dram_tensor("wp", [512, D], BF, kind="ExternalInput")
    ident = nc.dram_tensor("ident", [128, 128], BF, kind="ExternalInput")
    y = nc.dram_tensor("y", [T, D], BF, kind="ExternalOutput")

    with tile.TileContext(nc) as tc:
        with (
            tc.tile_pool(name="pers", bufs=1) as pers,
            tc.tile_pool(name="work", bufs=1) as work,
            tc.tile_pool(name="ps", bufs=1, space="PSUM") as pp,
        ):
            # ---- persistent SBUF (per-dc tiles => DMA-granular deps) ----
            # xT split by (d-chunk, token-quarter) so q-group 0 compute can
            # start as soon as the first quarter lands
            xT_t = [[pers.tile([128, 512], BF, tag=f"xT{dc}_{tq}",
                               name=f"xT{dc}_{tq}") for tq in range(4)]
                    for dc in range(8)]
            wq_t = [pers.tile([128, 512], BF, tag=f"wq{dc}", name=f"wq{dc}")
                    for dc in range(8)]
            wk_t = [pers.tile([128, 512], BF, tag=f"wk{dc}", name=f"wk{dc}")
                    for dc in range(8)]
            wv_t = [pers.tile([128, 512], BF, tag=f"wv{dc}", name=f"wv{dc}")
                    for dc in range(8)]
            wp_sb = pers.tile([128, 4, D], BF, tag="wp", name="wp_sb")
            id_sb = pers.tile([128, 128], BF, tag="id", name="id_sb")
            # V in token-major layout with a ones column per head: [tok, head, 65]
            vext = pers.tile([128, NKC, HPC, 65], BF, tag="vext", name="vext")
            # normalized attention outputs, d-major: [pair-chan, pair, tok]
            outT = pers.tile([128, PAIRS, T], BF, tag="outT", name="outT")
            # causal mask for diagonal blocks: keep q >= k
            mask_sb = pers.tile([128, 128], BF, tag="mask", name="mask_sb")

            # ---- loads, chunk-interleaved so compute starts early ----
            nc.sync.dma_start(id_sb[:, :], ident[:, :])
            for dc in range(8):
                nc.sync.dma_start(wq_t[dc][:, :], wq[dc * 128:(dc + 1) * 128, :])
                nc.sync.dma_start(wk_t[dc][:, :], wk[dc * 128:(dc + 1) * 128, :])
            for dc in range(8):
                nc.sync.dma_start(
                    xT_t[dc][0][:, :], xT[dc * 128:(dc + 1) * 128, 0:512])
                nc.sync.dma_start(wv_t[dc][:, :], wv[dc * 128:(dc + 1) * 128, :])
            for tq in range(1, 4):
                for dc in range(8):
                    nc.sync.dma_start(
                        xT_t[dc][tq][:, :],
                        xT[dc * 128:(dc + 1) * 128, tq * 512:(tq + 1) * 512])
            for cc in range(4):
                nc.sync.dma_start(wp_sb[:, cc, :], wp[cc * 128:(cc + 1) * 128, :])
            nc.gpsimd.memset(vext[:, :, :, 64], 1.0)
            nc.gpsimd.memset(mask_sb[:, :], 1.0)
            nc.gpsimd.affine_select(
                mask_sb[:, :],
                mask_sb[:, :],
                pattern=[[1, 128]],
                compare_op=mybir.AluOpType.is_ge,
                fill=0.0,
                base=0,
                channel_multiplier=-1,
            )

            # ---- background work pieces (V proj, Q/K proj) ----
            pair_qt = {}
            pair_kt = {}
            v_done = set()
            proj_done = set()

            def emit_v(tk):
                """V chunk tk: vext[:, tk] = (x @ wv)[tk block], + ones col."""
                if tk in v_done:
                    return
                v_done.add(tk)
                ps_v = pp.tile([128, 512], F32, tag="pq", bufs=1, name="ps_v")
                tq, to = tk // 4, (tk % 4) * 128
                for dc in range(8):
                    nc.tensor.matmul(
                        ps_v[:, :],
                        xT_t[dc][tq][:, to:to + 128],
                        wv_t[dc][:, :],
                        start=(dc == 0),
                        stop=(dc == 7),
                        skip_group_check=True,
                    )
                nc.vector.tensor_copy(
                    vext[:, tk, :, 0:64],
                    ps_v.rearrange("p (h d) -> p h d", d=64),
                )

            def emit_proj(hp, qg, which):
                """Q^T or K^T for (pair hp, q-group qg), d-major.

                Rows = pair channels: head0 d 0-63 on partitions 0-63,
                head1 d 0-63 on partitions 64-127.
                """
                if (hp, qg, which) in proj_done:
                    return
                proj_done.add((hp, qg, which))
                if hp not in pair_qt:
                    pair_qt[hp] = work.tile([128, T], BF, tag="qt", bufs=2,
                                            name="qt")
                    pair_kt[hp] = work.tile([128, T], BF, tag="kt", bufs=2,
                                            name="kt")
                dst = pair_qt[hp] if which == "q" else pair_kt[hp]
                w_t = wq_t if which == "q" else wk_t
                ps_p = pp.tile([128, 512], F32, tag="pq", bufs=1, name="ps_p")
                for dc in range(8):
                    nc.tensor.matmul(
                        ps_p[:, :],
                        w_t[dc][:, hp * 128:(hp + 1) * 128],
                        xT_t[dc][qg][:, :],
                        start=(dc == 0),
                        stop=(dc == 7),
                        skip_group_check=True,
                    )
                nc.vector.tensor_copy(dst[:, qg * TQ:(qg + 1) * TQ], ps_p[:, :])

            queue = []

            def pump(n):
                for _ in range(min(n, len(queue))):
                    queue.pop(0)()

            # ---- phase 2 ----
            emit_proj(0, 0, "q")
            emit_proj(0, 0, "k")

            for hp in range(PAIRS):
                for qg in range(NQG):
                    kmax = (qg + 1) * QB
                    noff = qg * QB
                    # overdue pieces first (deps of this q-group)
                    pump(len(queue))
                    emit_proj(hp, qg, "q")
                    emit_proj(hp, qg, "k")
                    # enqueue pieces for the next q-group
                    if qg + 1 < NQG:
                        nhp, nqg = hp, qg + 1
                    elif hp + 1 < PAIRS:
                        nhp, nqg = hp + 1, 0
                    else:
                        nhp = None
                    if nhp is not None:
                        queue.append(lambda a=nhp, b=nqg: emit_proj(a, b, "q"))
                        queue.append(lambda a=nhp, b=nqg: emit_proj(a, b, "k"))
                        if nhp == 0:
                            for tk in range(kmax, (nqg + 1) * QB):
                                queue.append(lambda t=tk: emit_v(t))

                    qt, kt = pair_qt[hp], pair_kt[hp]
                    # unnormalized AV accumulators, q-major:
                    # region (h, qb) = psO[h][:, qb, 0:64] + den col 64
                    psO = [
                        pp.tile([128, QB, 65], F32, tag=f"av{h}", bufs=1,
                                name=f"psO{h}")
                        for h in range(2)
                    ]
                    # transposed normalized outputs [d, (h qb), q]
                    psT = pp.tile([64, 2 * QB, 128], BF, tag="tp", bufs=1,
                                  name="psT")

                    def qk(kc):
                        # scores^T chunk for both heads: [k 128, q 512] x2
                        # on diagonal blocks only columns q >= j*128 live
                        off = max(0, kc - noff) * TK
                        ps_s = pp.tile([128, 1024], F32, tag="sc", bufs=2,
                                       name="ps_s")
                        for h in (0, 1):
                            nc.tensor.matmul(
                                ps_s[:, h * 512 + off:(h + 1) * 512],
                                kt[h * 64:(h + 1) * 64, kc * TK:(kc + 1) * TK],
                                qt[h * 64:(h + 1) * 64,
                                   qg * TQ + off:(qg + 1) * TQ],
                                start=True, stop=True,
                                skip_group_check=True,
                            )
                        return ps_s

                    def evict(h, qb):
                        # normalize region (h, qb) and transpose to psT
                        den_r = work.tile([128, 1], F32, tag="denr", bufs=4,
                                          name="den_r")
                        nc.vector.reciprocal_approx_fast(
                            den_r[:, :], psO[h][:, qb, 64:65])
                        o_sb = work.tile([128, 64], BF, tag="osb", bufs=4,
                                         name="o_sb")
                        nc.vector.tensor_scalar_mul(
                            o_sb[:, :], psO[h][:, qb, 0:64], den_r[:, :])
                        nc.tensor.transpose(
                            psT[0:64, h * QB + qb, :], o_sb[:, :], id_sb[:, :])

                    def softmax_av(kc, ps_s):
                        off = max(0, kc - noff) * TK
                        j = kc - noff
                        ex = work.tile([128, 2, 512], BF, tag="ex", bufs=4,
                                       name="ex")
                        ps3 = ps_s.rearrange("p (h q) -> p h q", h=2)
                        nc.scalar.activation(
                            ex[:, :, off:], ps3[:, :, off:],
                            mybir.ActivationFunctionType.Exp,
                        )
                        if hp == 0:
                            # just-in-time V chunks, one chunk ahead
                            emit_v(kc)
                            if kc + 1 < kmax:
                                emit_v(kc + 1)
                        if j >= 0:
                            # causal mask on the diagonal 128x128 sub-block
                            for h in (0, 1):
                                nc.vector.tensor_mul(
                                    ex[:, h, off:off + TK],
                                    ex[:, h, off:off + TK],
                                    mask_sb[:, :],
                                )
                        for h in (0, 1):
                            for qb in range(QB):
                                if j > qb:
                                    continue
                                # start=True clears has_written for the WHOLE
                                # psum bank, so only the first matmul per bank
                                # may set it; later first-writes of other
                                # regions overwrite via has_written=0.
                                nc.tensor.matmul(
                                    psO[h][:, qb, :],
                                    ex[:, h, qb * TK:(qb + 1) * TK],
                                    vext[:, kc, hp * 2 + h, :],
                                    start=(kc == 0 and qb == 0),
                                    stop=(kc == noff + qb),
                                    skip_group_check=True,
                                )
                        if j >= 0:
                            for h in (0, 1):
                                evict(h, j)

                    prev = qk(0)
                    for kc in range(kmax):
                        nxt = qk(kc + 1) if kc + 1 < kmax else None
                        softmax_av(kc, prev)
                        pump(1)
                        prev = nxt

                    for h in (0, 1):
                        nc.vector.tensor_copy(
                            outT[h * 64:(h + 1) * 64, hp,
                                 qg * TQ:(qg + 1) * TQ],
                            psT[0:64, h * QB:(h + 1) * QB, :],
                        )

            # ---- phase 3: y_partial = outT.T @ wp ----
            for tk in range(NKC):
                ps_y = pp.tile([128, 1024], F32, tag="sc", bufs=2, name="ps_y")
                for nb in range(2):
                    for cc in range(4):
                        nc.tensor.matmul(
                            ps_y[:, nb * 512:(nb + 1) * 512],
                            outT[:, cc, tk * 128:(tk + 1) * 128],
                            wp_sb[:, cc, nb * 512:(nb + 1) * 512],
                            start=(cc == 0),
                            stop=(cc == 3),
                            skip_group_check=True,
                        )
                y_ev = work.tile([128, 1024], BF, tag="yev", bufs=3,
                                 name="y_ev")
                # alternate eviction engines (both idle in phase 3)
                for nb in range(2):
                    sl = slice(nb * 512, (nb + 1) * 512)
                    if (tk + nb) % 2 == 0:
                        nc.vector.tensor_copy(y_ev[:, sl], ps_y[:, sl])
                    else:
                        nc.scalar.copy(y_ev[:, sl], ps_y[:, sl])
                    nc.sync.dma_start(
                        y[tk * 128:(tk + 1) * 128, sl],
                        y_ev[:, sl],
                    )

    nc.compile()
    return nc


_NC_CACHE = None


def _get_nc():
    global _NC_CACHE
    if _NC_CACHE is None:
        _NC_CACHE = build_nc()
    return _NC_CACHE


def make_in_maps(x, w_qkv, w_proj):
    """Host-side sharding: core c -> (batch c//2, head-group c%2)."""
    scale = np.float32(1.0 / np.sqrt(DH))
    ident = np.eye(128, dtype=BF_NP)
    in_maps = []
    for c in range(N_CORES):
        b, g = divmod(c, 2)
        sl = slice(g * 512, (g + 1) * 512)
        xT = np.ascontiguousarray(x[b].T).astype(BF_NP)
        wq = (w_qkv[:, 0 * D:1 * D][:, sl] * scale).astype(BF_NP)
        wk = w_qkv[:, 1 * D:2 * D][:, sl].astype(BF_NP)
        wv = w_qkv[:, 2 * D:3 * D][:, sl].astype(BF_NP)
        wp = np.ascontiguousarray(w_proj[sl, :]).astype(BF_NP)
        in_maps.append({"xT": xT, "wq": wq, "wk": wk, "wv": wv, "wp": wp,
                        "ident": ident})
    return in_maps


def kernel(x, w_qkv, w_proj, _trace=False, _tmpdir=None):
    x = np.asarray(x, dtype=np.float32)
    w_qkv = np.asarray(w_qkv, dtype=np.float32)
    w_proj = np.asarray(w_proj, dtype=np.float32)
    nc = _get_nc()
    in_maps = make_in_maps(x, w_qkv, w_proj)
    res = run_bass_kernel_spmd(
        nc, in_maps, core_ids=list(range(N_CORES)), trace=_trace, tmpdir=_tmpdir
    )
    out = np.empty((B, T, D), dtype=np.float32)
    for b in range(B):
        out[b] = (res.results[2 * b]["y"].astype(np.float32)
                  + res.results[2 * b + 1]["y"].astype(np.float32))
    if _trace:
        kernel._last_results = res
    return out


# revision 44
# speedup vs baseline: 1.3274x; 1.0191x over previous
"""Causal multi-head attention (B=4, T=2048, D=1024, H=16) on 8 trn2 cores.

Sharding: core c -> (batch b = c//2, head-group g = c%2) -> 8 heads/core.
Per-core Bass kernel: QKV projections, causal flash attention with
transposed scores (s^T = K @ Q^T) but q-major AV accumulation
(out[q, d] = ex^T V via ex-as-stationary matmuls, N=65 with an appended
ones column in V giving the softmax denominator per psum partition).
Normalization folds into PSUM eviction (reciprocal_approx_fast +
per-partition tensor_scalar_mul), then a PE transpose restores d-major
layout for the output projection. The attention inner loop is ACT
(exp) throughput bound, so V projection and Q/K projections are diced
into small matmul "pieces" pumped into the PE's slack between chunks.
Host sums the two head-group partials per batch (row-parallel proj).
"""

import numpy as np
import ml_dtypes

import concourse.bass as bass  # noqa: F401  (bass types via bacc)
import concourse.bacc as bacc
import concourse.mybir as mybir
import concourse.tile as tile
from concourse.bass_utils import run_bass_kernel_spmd

B, T, D = 4, 2048, 1024
H, DH = 16, 64
N_CORES = 8
HPC = 8      # heads per core
PAIRS = HPC // 2
BF = mybir.dt.bfloat16
F32 = mybir.dt.float32
BF_NP = ml_dtypes.bfloat16

TQ = 512     # q block (free dim)
TK = 128     # k block (partition dim)
NQG = T // TQ
NKC = T // TK
QB = TQ // TK   # 128-wide q sub-blocks per q group


def build_nc():
    nc = bacc.Bacc(
        "TRN2",
        target_bir_lowering=False,
        debug=False,
        enable_asserts=True,
        num_devices=N_CORES,
    )
    xT = nc.dram_tensor("xT", [D, T], BF, kind="ExternalInput")
    wq = nc.dram_tensor("wq", [D, 512], BF, kind="ExternalInput")
    wk = nc.dram_tensor("wk", [D, 512], BF, kind="ExternalInput")
    wv = nc.dram_tensor("wv", [D, 512], BF, kind="ExternalInput")
    wp = nc.dram_tensor("wp", [512, D], BF, kind="ExternalInput")
    ident = nc.dram_tensor("ident", [128, 128], BF, kind="ExternalInput")
    y = nc.dram_tensor("y", [T, D], BF, kind="ExternalOutput")

    with tile.TileContext(nc) as tc:
        with (
            tc.tile_pool(name="pers", bufs=1) as pers,
            tc.tile_pool(name="work", bufs=1) as work,
            tc.tile_pool(name="ps", bufs=1, space="PSUM") as pp,
        ):
            # ---- persistent SBUF (per-dc tiles => DMA-granular deps) ----
            # xT split by (d-chunk, token-quarter) so q-group 0 compute can
            # start as soon as the first quarter lands
            xT_t = [[pers.tile([128, 512], BF, tag=f"xT{dc}_{tq}",
                               name=f"xT{dc}_{tq}") for tq in range(4)]
                    for dc in range(8)]
            wq_t = [pers.tile([128, 512], BF, tag=f"wq{dc}", name=f"wq{dc}")
                    for dc in range(8)]
            wk_t = [pers.tile([128, 512], BF, tag=f"wk{dc}", name=f"wk{dc}")
                    for dc in range(8)]
            wv_t = [pers.tile([128, 512], BF, tag=f"wv{dc}", name=f"wv{dc}")
                    for dc in range(8)]
            wp_sb = pers.tile([128, 4, D], BF, tag="wp", name="wp_sb")
            id_sb = pers.tile([128, 128], BF, tag="id", name="id_sb")
            # V in token-major layout with a ones column per head: [tok, head, 65]
            vext = pers.tile([128, NKC, HPC, 65], BF, tag="vext", name="vext")
            # normalized attention outputs, d-major: [pair-chan, pair, tok]
            outT = pers.tile([128, PAIRS, T], BF, tag="outT", name="outT")
            # causal mask for diagonal blocks: keep q >= k
            mask_sb = pers.tile([128, 128], BF, tag="mask", name="mask_sb")

            # ---- loads, chunk-interleaved so compute starts early ----
            # spread DMA triggers across engine queues (trigger issue is the
            # serial bottleneck, ~0.6us per dma_start on one queue)
            nc.sync.dma_start(id_sb[:, :], ident[:, :])
            for dc in range(8):
                nc.scalar.dma_start(wq_t[dc][:, :],
                                    wq[dc * 128:(dc + 1) * 128, :])
                nc.sync.dma_start(wk_t[dc][:, :],
                                  wk[dc * 128:(dc + 1) * 128, :])
                nc.gpsimd.dma_start(
                    xT_t[dc][0][:, :], xT[dc * 128:(dc + 1) * 128, 0:512])
            for dc in range(8):
                nc.scalar.dma_start(wv_t[dc][:, :],
                                    wv[dc * 128:(dc + 1) * 128, :])
                nc.gpsimd.dma_start(
                    xT_t[dc][1][:, :], xT[dc * 128:(dc + 1) * 128, 512:1024])
            for tq in range(2, 4):
                for dc in range(8):
                    eng = nc.sync if tq == 2 else nc.gpsimd
                    eng.dma_start(
                        xT_t[dc][tq][:, :],
                        xT[dc * 128:(dc + 1) * 128, tq * 512:(tq + 1) * 512])
            for cc in range(4):
                nc.sync.dma_start(wp_sb[:, cc, :],
                                  wp[cc * 128:(cc + 1) * 128, :])
            nc.gpsimd.memset(vext[:, :, :, 64], 1.0)
            nc.gpsimd.memset(mask_sb[:, :], 1.0)
            nc.gpsimd.affine_select(
                mask_sb[:, :],
                mask_sb[:, :],
                pattern=[[1, 128]],
                compare_op=mybir.AluOpType.is_ge,
                fill=0.0,
                base=0,
                channel_multiplier=-1,
            )

            # ---- background work pieces (V proj, Q/K proj) ----
            pair_qt = {}
            pair_kt = {}
            v_done = set()
            proj_done = set()

            def emit_v(tk):
                """V chunk tk: vext[:, tk] = (x @ wv)[tk block], + ones col."""
                if tk in v_done:
                    return
                v_done.add(tk)
                ps_v = pp.tile([128, 512], F32, tag="pq", bufs=1, name="ps_v")
                tq, to = tk // 4, (tk % 4) * 128
                for dc in range(8):
                    nc.tensor.matmul(
                        ps_v[:, :],
                        xT_t[dc][tq][:, to:to + 128],
                        wv_t[dc][:, :],
                        start=(dc == 0),
                        stop=(dc == 7),
                        skip_group_check=True,
                    )
                nc.vector.tensor_copy(
                    vext[:, tk, :, 0:64],
                    ps_v.rearrange("p (h d) -> p h d", d=64),
                )

            def emit_proj(hp, qg, which):
                """Q^T or K^T for (pair hp, q-group qg), d-major.

                Rows = pair channels: head0 d 0-63 on partitions 0-63,
                head1 d 0-63 on partitions 64-127.
                """
                if (hp, qg, which) in proj_done:
                    return
                proj_done.add((hp, qg, which))
                if hp not in pair_qt:
                    pair_qt[hp] = work.tile([128, T], BF, tag="qt", bufs=2,
                                            name="qt")
                    pair_kt[hp] = work.tile([128, T], BF, tag="kt", bufs=2,
                                            name="kt")
                dst = pair_qt[hp] if which == "q" else pair_kt[hp]
                w_t = wq_t if which == "q" else wk_t
                ps_p = pp.tile([128, 512], F32, tag="pq", bufs=1, name="ps_p")
                for dc in range(8):
                    nc.tensor.matmul(
                        ps_p[:, :],
                        w_t[dc][:, hp * 128:(hp + 1) * 128],
                        xT_t[dc][qg][:, :],
                        start=(dc == 0),
                        stop=(dc == 7),
                        skip_group_check=True,
                    )
                nc.vector.tensor_copy(dst[:, qg * TQ:(qg + 1) * TQ], ps_p[:, :])

            queue = []

            def pump(n):
                for _ in range(min(n, len(queue))):
                    queue.pop(0)()

            # ---- phase 2 ----
            emit_proj(0, 0, "q")
            emit_proj(0, 0, "k")

            for hp in range(PAIRS):
                for qg in range(NQG):
                    kmax = (qg + 1) * QB
                    noff = qg * QB
                    # overdue pieces first (deps of this q-group)
                    pump(len(queue))
                    emit_proj(hp, qg, "q")
                    emit_proj(hp, qg, "k")
                    # enqueue pieces for the next q-group
                    if qg + 1 < NQG:
                        nhp, nqg = hp, qg + 1
                    elif hp + 1 < PAIRS:
                        nhp, nqg = hp + 1, 0
                    else:
                        nhp = None
                    if nhp is not None:
                        queue.append(lambda a=nhp, b=nqg: emit_proj(a, b, "q"))
                        queue.append(lambda a=nhp, b=nqg: emit_proj(a, b, "k"))
                        if nhp == 0:
                            for tk in range(kmax, (nqg + 1) * QB):
                                queue.append(lambda t=tk: emit_v(t))

                    qt, kt = pair_qt[hp], pair_kt[hp]
                    # unnormalized AV accumulators, q-major:
                    # region (h, qb) = psO[h][:, qb, 0:64] + den col 64
                    psO = [
                        pp.tile([128, QB, 65], F32, tag=f"av{h}", bufs=1,
                                name=f"psO{h}")
                        for h in range(2)
                    ]
                    # transposed normalized outputs [d, (h qb), q]
                    psT = pp.tile([64, 2 * QB, 128], BF, tag="tp", bufs=1,
                                  name="psT")

                    def qk(kc):
                        # scores^T chunk for both heads: [k 128, q 512] x2
                        # on diagonal blocks only columns q >= j*128 live
                        off = max(0, kc - noff) * TK
                        ps_s = pp.tile([128, 1024], F32, tag="sc", bufs=2,
                                       name="ps_s")
                        for h in (0, 1):
                            nc.tensor.matmul(
                                ps_s[:, h * 512 + off:(h + 1) * 512],
                                kt[h * 64:(h + 1) * 64, kc * TK:(kc + 1) * TK],
                                qt[h * 64:(h + 1) * 64,
                                   qg * TQ + off:(qg + 1) * TQ],
                                start=True, stop=True,
                                skip_group_check=True,
                            )
                        return ps_s

                    def evict(h, qb):
                        # normalize region (h, qb) and transpose to psT
                        den_r = work.tile([128, 1], F32, tag="denr", bufs=4,
                                          name="den_r")
                        nc.vector.reciprocal_approx_fast(
                            den_r[:, :], psO[h][:, qb, 64:65])
                        o_sb = work.tile([128, 64], BF, tag="osb", bufs=4,
                                         name="o_sb")
                        nc.vector.tensor_scalar_mul(
                            o_sb[:, :], psO[h][:, qb, 0:64], den_r[:, :])
                        nc.tensor.transpose(
                            psT[0:64, h * QB + qb, :], o_sb[:, :], id_sb[:, :])

                    def softmax_av(kc, ps_s):
                        off = max(0, kc - noff) * TK
                        j = kc - noff
                        ex = work.tile([128, 2, 512], BF, tag="ex", bufs=4,
                                       name="ex")
                        ps3 = ps_s.rearrange("p (h q) -> p h q", h=2)
                        nc.scalar.activation(
                            ex[:, :, off:], ps3[:, :, off:],
                            mybir.ActivationFunctionType.Exp,
                        )
                        if hp == 0:
                            # just-in-time V chunks, one chunk ahead
                            emit_v(kc)
                            if kc + 1 < kmax:
                                emit_v(kc + 1)
                        if j >= 0:
                            # causal mask on the diagonal 128x128 sub-block
                            for h in (0, 1):
                                nc.vector.tensor_mul(
                                    ex[:, h, off:off + TK],
                                    ex[:, h, off:off + TK],
                                    mask_sb[:, :],
                                )
                        for h in (0, 1):
                            for qb in range(QB):
                                if j > qb:
                                    continue
                                # start=True clears has_written for the WHOLE
                                # psum bank, so only the first matmul per bank
                                # may set it; later first-writes of other
                                # regions overwrite via has_written=0.
                                nc.tensor.matmul(
                                    psO[h][:, qb, :],
                                    ex[:, h, qb * TK:(qb + 1) * TK],
                                    vext[:, kc, hp * 2 + h, :],
                                    start=(kc == 0 and qb == 0),
                                    stop=(kc == noff + qb),
                                    skip_group_check=True,
                                )
                        if j >= 0:
                            for h in (0, 1):
                                evict(h, j)

                    prev = qk(0)
                    for kc in range(kmax):
                        nxt = qk(kc + 1) if kc + 1 < kmax else None
                        softmax_av(kc, prev)
                        pump(1)
                        prev = nxt

                    for h in (0, 1):
                        nc.vector.tensor_copy(
                            outT[h * 64:(h + 1) * 64, hp,
                                 qg * TQ:(qg + 1) * TQ],
                            psT[0:64, h * QB:(h + 1) * QB, :],
                        )

            # ---- phase 3: y_partial = outT.T @ wp ----
            for tk in range(NKC):
                ps_y = pp.tile([128, 1024], F32, tag="sc", bufs=2, name="ps_y")
                for nb in range(2):
                    for cc in range(4):
                        nc.tensor.matmul(
                            ps_y[:, nb * 512:(nb + 1) * 512],
                            outT[:, cc, tk * 128:(tk + 1) * 128],
                            wp_sb[:, cc, nb * 512:(nb + 1) * 512],
                            start=(cc == 0),
                            stop=(cc == 3),
                            skip_group_check=True,
                        )
                y_ev = work.tile([128, 1024], BF, tag="yev", bufs=3,
                                 name="y_ev")
                # alternate eviction engines (both idle in phase 3)
                for nb in range(2):
                    sl = slice(nb * 512, (nb + 1) * 512)
                    if (tk + nb) % 2 == 0:
                        nc.vector.tensor_copy(y_ev[:, sl], ps_y[:, sl])
                        nc.sync.dma_start(
                            y[tk * 128:(tk + 1) * 128, sl], y_ev[:, sl])
                    else:
                        nc.scalar.copy(y_ev[:, sl], ps_y[:, sl])
                        nc.gpsimd.dma_start(
                            y[tk * 128:(tk + 1) * 128, sl], y_ev[:, sl])

    nc.compile()
    return nc


_NC_CACHE = None


def _get_nc():
    global _NC_CACHE
    if _NC_CACHE is None:
        _NC_CACHE = build_nc()
    return _NC_CACHE


def make_in_maps(x, w_qkv, w_proj):
    """Host-side sharding: core c -> (batch c//2, head-group c%2)."""
    scale = np.float32(1.0 / np.sqrt(DH))
    ident = np.eye(128, dtype=BF_NP)
    in_maps = []
    for c in range(N_CORES):
        b, g = divmod(c, 2)
        sl = slice(g * 512, (g + 1) * 512)
        xT = np.ascontiguousarray(x[b].T).astype(BF_NP)
        wq = (w_qkv[:, 0 * D:1 * D][:, sl] * scale).astype(BF_NP)
        wk = w_qkv[:, 1 * D:2 * D][:, sl].astype(BF_NP)
        wv = w_qkv[:, 2 * D:3 * D][:, sl].astype(BF_NP)
        wp = np.ascontiguousarray(w_proj[sl, :]).astype(BF_NP)
        in_maps.append({"xT": xT, "wq": wq, "wk": wk, "wv": wv, "wp": wp,
                        "ident": ident})
    return in_maps


def kernel(x, w_qkv, w_proj, _trace=False, _tmpdir=None):
    x = np.asarray(x, dtype=np.float32)
    w_qkv = np.asarray(w_qkv, dtype=np.float32)
    w_proj = np.asarray(w_proj, dtype=np.float32)
    nc = _get_nc()
    in_maps = make_in_maps(x, w_qkv, w_proj)
    res = run_bass_kernel_spmd(
        nc, in_maps, core_ids=list(range(N_CORES)), trace=_trace, tmpdir=_tmpdir
    )
    out = np.empty((B, T, D), dtype=np.float32)
    for b in range(B):
        out[b] = (res.results[2 * b]["y"].astype(np.float32)
                  + res.results[2 * b + 1]["y"].astype(np.float32))
    if _trace:
        kernel._last_results = res
    return out
